# revision 1
# baseline (speedup 1.0000x reference)
"""Graphwise KL loss (segment_reduce) on 8 trn2 NeuronCores.

Strategy:
  Device (the O(N) memory-bound work, data-parallel over 8 cores, each core
  streams a contiguous 1/8 slice of the element arrays):
    pr = y_true * weight
    e1 = pr * (ln(pr + 1e-37) - ln(y_pred + 1e-8))
    out: 32-element block sums of e1 and pr        (2 x 32768 f32 per core)
  Host (O(num_graphs) metadata assembly, fp64):
    Per-segment sums A_g (of e1) and B_g (of pr) are reconstructed from the
    device block sums plus fp64 partial sums of the (< 32-element) block
    prefixes at each segment boundary.  With S_g = max(B_g, EPS):
      total = mean_g (A_g - B_g * ln(S_g)) / S_g
    which equals the reference's  sum_g sum_i p*(ln p - ln q)  with
    p = pr/S_g  (identical up to the ln(max(p,EPS)) clip on the ~1e2
    elements with p < 1e-8, which contribute O(1e-7) relative error).

  Raw Bass (no Tile): this walrus build caps every non-EventSemaphore
  instruction at ONE inline sync wait, so all waits are standalone wait_ge
  instructions and all cross-engine sync is explicit, with double-buffered
  tiles (buf = t % 2) and per-engine instruction streams.
"""

import numpy as np

N_TOTAL = 8388608
N_CORES = 8
N_LOCAL = N_TOTAL // N_CORES      # 1048576
P = 128
TILE_F = 2048                     # free dim of one macro tile
TILE_ELEMS = P * TILE_F           # 262144
N_TILES = N_LOCAL // TILE_ELEMS   # 4
BLK = 32
JPT = TILE_F // BLK               # 64 block sums per partition per tile
N_BLOCKS_LOCAL = N_LOCAL // BLK   # 32768
EPS = 1e-8
TINY = 1e-37

_CACHE = {}


def _check_one_wait(nc):
    """Assert no non-EventSemaphore instruction carries more than one wait."""
    bad = []
    for f in nc.m.functions:
        for bb in f.blocks:
            for inst in bb.instructions:
                si = inst.sync_info
                if si and si.on_wait and len(si.on_wait) > 1:
                    if "EventSem" not in type(inst).__name__:
                        bad.append((type(inst).__name__, inst.name, len(si.on_wait)))
    assert not bad, f"multi-wait instructions remain: {bad}"


def _build_program():
    import concourse.bass as bass
    import concourse.mybir as mybir

    f32 = mybir.dt.float32
    Ln = mybir.ActivationFunctionType.Ln
    X = mybir.AxisListType.X
    ADD = mybir.AluOpType.add

    nc = bass.Bass()

    # Const APs for the Ln biases (same mechanism Bass.__init__ uses for 0/1).
    for val in (TINY, EPS):
        ct = nc.alloc_sbuf_tensor(f"const-f32-{val}", [128, 1], f32)
        nc.gpsimd.memset(ct.ap(), val)
        nc.const_aps.aps[(f32, val)] = ct.ap()
    nc.all_engine_barrier()

    yp = nc.declare_dram_parameter("yp", [N_LOCAL], f32, isOutput=False)
    yt = nc.declare_dram_parameter("yt", [N_LOCAL], f32, isOutput=False)
    w = nc.declare_dram_parameter("w", [N_LOCAL], f32, isOutput=False)
    o1 = nc.declare_dram_parameter("o1", [N_BLOCKS_LOCAL], f32, isOutput=True)
    o2 = nc.declare_dram_parameter("o2", [N_BLOCKS_LOCAL], f32, isOutput=True)

    yp3 = yp[:].rearrange("(t p f) -> t p f", p=P, f=TILE_F)
    yt3 = yt[:].rearrange("(t p f) -> t p f", p=P, f=TILE_F)
    w3 = w[:].rearrange("(t p f) -> t p f", p=P, f=TILE_F)
    o13 = o1[:].rearrange("(t p j) -> t p j", p=P, j=JPT)
    o23 = o2[:].rearrange("(t p j) -> t p j", p=P, j=JPT)

    # Double-buffered SBUF tiles.
    def buf2(name, shape):
        return [nc.alloc_sbuf_tensor(f"{name}{i}", shape, f32).ap() for i in range(2)]

    t_yp = buf2("t_yp", [P, TILE_F])
    t_yt = buf2("t_yt", [P, TILE_F])
    t_w = buf2("t_w", [P, TILE_F])
    t_pr = buf2("t_pr", [P, TILE_F])
    t_lp = buf2("t_lp", [P, TILE_F])
    t_lq = buf2("t_lq", [P, TILE_F])
    t_d = buf2("t_d", [P, TILE_F])
    t_e1 = buf2("t_e1", [P, TILE_F])
    t_b1 = buf2("t_b1", [P, JPT])
    t_b2 = buf2("t_b2", [P, JPT])

    # Even/odd semaphores per DMA stream: at most ONE DMA in flight per sem,
    # so its 16 completion sub-increments can't interleave with another
    # transfer's (CoreSim SemaphoreRace otherwise).
    s_yp = [nc.alloc_semaphore(f"s_yp{i}") for i in range(2)]  # +16 per load
    s_yt = [nc.alloc_semaphore(f"s_yt{i}") for i in range(2)]
    s_w = [nc.alloc_semaphore(f"s_w{i}") for i in range(2)]
    s_out = [nc.alloc_semaphore(f"s_out{i}") for i in range(2)]  # +32 per iter
    s_act = nc.alloc_semaphore("s_act")  # +1 per ACT op (lp, lq per iter)
    s_dve = nc.alloc_semaphore("s_dve")  # +1 per DVE op

    # DVE op order (hoisted pr for cross-engine overlap):
    #   pr(0), pr(1), [d,e1,r1,r2](0), pr(2), [d,e1,r1,r2](1), pr(3),
    #   [d,e1,r1,r2](2), [d,e1,r1,r2](3)
    # Absolute DVE indices (1-based):
    dve_idx = {}
    n = 0
    order = [("pr", 0), ("pr", 1)]
    for t in range(N_TILES):
        order.append(("blk", t))
        if t + 2 < N_TILES:
            order.append(("pr", t + 2))
    for item in order:
        kind, t = item
        if kind == "pr":
            n += 1
            dve_idx[("pr", t)] = n
        else:
            for opname in ("d", "e1", "r1", "r2"):
                n += 1
                dve_idx[(opname, t)] = n
    n_dve_total = n

    with nc.Block() as block:

        @block.gpsimd
        def _(g):
            for t in range(N_TILES):
                if t >= 2:
                    # typ[buf] was read by lq(t-2) = ACT op 2(t-2)+2
                    g.wait_ge(s_act, 2 * (t - 2) + 2)
                    # tyt/tw[buf] read by pr(t-2); b-out wait below covers DVE
                    g.wait_ge(s_dve, dve_idx[("pr", t - 2)])
                buf = t % 2
                g.dma_start(t_yp[buf], yp3[t, :, :]).then_inc(s_yp[buf], 16)
                g.dma_start(t_yt[buf], yt3[t, :, :]).then_inc(s_yt[buf], 16)
                g.dma_start(t_w[buf], w3[t, :, :]).then_inc(s_w[buf], 16)
                if t >= 1:
                    # store iteration t-1 outputs
                    tt = t - 1
                    g.wait_ge(s_dve, dve_idx[("r2", tt)])
                    g.dma_start(o13[tt, :, :], t_b1[tt % 2]).then_inc(s_out[tt % 2], 16)
                    g.dma_start(o23[tt, :, :], t_b2[tt % 2]).then_inc(s_out[tt % 2], 16)
            tt = N_TILES - 1
            g.wait_ge(s_dve, dve_idx[("r2", tt)])
            g.dma_start(o13[tt, :, :], t_b1[tt % 2]).then_inc(s_out[tt % 2], 16)
            g.dma_start(o23[tt, :, :], t_b2[tt % 2]).then_inc(s_out[tt % 2], 16)
            # ensure all stores landed before program end
            for i in range(2):
                g.wait_ge(s_out[i], 32 * (N_TILES // 2))

        @block.scalar
        def _(s):
            for t in range(N_TILES):
                buf = t % 2
                # lp(t) = Ln(pr(t) + TINY): needs DVE pr(t); also covers
                # lp/lq[buf] slot reuse (d(t-2) precedes pr(t) in DVE order)
                s.wait_ge(s_dve, dve_idx[("pr", t)])
                s.activation(t_lp[buf], t_pr[buf], Ln, bias=TINY).then_inc(s_act, 1)
                # lq(t) = Ln(yp(t) + EPS)
                s.wait_ge(s_yp[buf], 16 * (t // 2 + 1))
                s.activation(t_lq[buf], t_yp[buf], Ln, bias=EPS).then_inc(s_act, 1)

        @block.vector
        def _(v):
            def emit_pr(t):
                buf = t % 2
                v.wait_ge(s_yt[buf], 16 * (t // 2 + 1))
                v.wait_ge(s_w[buf], 16 * (t // 2 + 1))
                v.tensor_mul(t_pr[buf], t_yt[buf], t_w[buf]).then_inc(s_dve, 1)

            def emit_blk(t):
                buf = t % 2
                v.wait_ge(s_act, 2 * t + 2)  # lp(t), lq(t) done
                v.tensor_sub(t_d[buf], t_lp[buf], t_lq[buf]).then_inc(s_dve, 1)
                # same-engine RAW: the DVE pipeline does not forward; an op
                # reading the previous op's output needs an explicit wait
                v.wait_ge(s_dve, dve_idx[("d", t)])
                v.tensor_mul(t_e1[buf], t_pr[buf], t_d[buf]).then_inc(s_dve, 1)
                if t >= 2:
                    # b1/b2[buf] were stored by out-DMAs of t-2
                    v.wait_ge(s_out[t % 2], 32 * ((t - 2) // 2 + 1))
                v.wait_ge(s_dve, dve_idx[("e1", t)])
                v.tensor_reduce(
                    t_b1[buf], t_e1[buf].rearrange("p (j b) -> p j b", b=BLK),
                    axis=X, op=ADD,
                ).then_inc(s_dve, 1)
                v.tensor_reduce(
                    t_b2[buf], t_pr[buf].rearrange("p (j b) -> p j b", b=BLK),
                    axis=X, op=ADD,
                ).then_inc(s_dve, 1)

            for item in order:
                if item[0] == "pr":
                    emit_pr(item[1])
                else:
                    emit_blk(item[1])

    _check_one_wait(nc)
    return nc


def _get_program():
    if "nc" not in _CACHE:
        _CACHE["nc"] = _build_program()
    return _CACHE["nc"]


def _run_device(yp, yt, w, trace=False):
    from concourse.bass_utils import run_bass_kernel_spmd

    nc = _get_program()
    in_maps = [
        {
            "yp": yp[k * N_LOCAL : (k + 1) * N_LOCAL],
            "yt": yt[k * N_LOCAL : (k + 1) * N_LOCAL],
            "w": w[k * N_LOCAL : (k + 1) * N_LOCAL],
        }
        for k in range(N_CORES)
    ]
    res = run_bass_kernel_spmd(nc, in_maps, list(range(N_CORES)), trace=trace)
    bs1 = np.concatenate([r["o1"].reshape(-1) for r in res.results])
    bs2 = np.concatenate([r["o2"].reshape(-1) for r in res.results])
    return bs1, bs2, res


def kernel(y_pred, y_true, weight, segment_ptr, _trace=False):
    yp = np.ascontiguousarray(np.asarray(y_pred), dtype=np.float32).reshape(-1)
    yt = np.ascontiguousarray(np.asarray(y_true), dtype=np.float32).reshape(-1)
    w = np.ascontiguousarray(np.asarray(weight), dtype=np.float32).reshape(-1)
    ptr = np.asarray(segment_ptr).astype(np.int64).reshape(-1)
    n = yp.shape[0]
    G = ptr.shape[0] - 1
    assert n == N_TOTAL, f"kernel compiled for N={N_TOTAL}, got {n}"

    bs1, bs2, res = _run_device(yp, yt, w, trace=_trace)
    _CACHE["last_res"] = res

    # ---- host assembly in fp64 ----
    pre1 = np.empty(bs1.shape[0] + 1)
    pre1[0] = 0.0
    np.cumsum(bs1, dtype=np.float64, out=pre1[1:])
    pre2 = np.empty(bs2.shape[0] + 1)
    pre2[0] = 0.0
    np.cumsum(bs2, dtype=np.float64, out=pre2[1:])

    # clip ptr defensively to [0, n] (reference guarantees this range)
    ptrc = np.clip(ptr, 0, n)
    b_idx = ptrc // BLK
    r = ptrc - b_idx * BLK  # offset within block
    # fp64 partial sums over [ptr - r, ptr) for boundaries not block-aligned
    seg_off = np.concatenate([[0], np.cumsum(r)])
    tot = int(seg_off[-1])
    part1 = np.zeros(ptrc.shape[0])
    part2 = np.zeros(ptrc.shape[0])
    if tot > 0:
        idx = np.repeat(ptrc - r, r) + (np.arange(tot) - np.repeat(seg_off[:-1], r))
        pr_h = yt[idx].astype(np.float64) * w[idx].astype(np.float64)
        e1_h = pr_h * (np.log(pr_h + TINY) - np.log(yp[idx].astype(np.float64) + EPS))
        nz = r > 0
        red_idx = np.minimum(seg_off[:-1][nz], tot - 1).astype(np.int64)
        part1[nz] = np.add.reduceat(e1_h, red_idx)
        part2[nz] = np.add.reduceat(pr_h, red_idx)

    C1 = pre1[b_idx] + part1
    C2 = pre2[b_idx] + part2
    A = np.diff(C1)
    Bg = np.diff(C2)
    S = np.maximum(Bg, EPS)
    total = np.sum((A - Bg * np.log(S)) / S) / max(G, 1)
    return np.float32(total)



# revision 2
# speedup vs baseline: 2.2051x; 2.2051x over previous
"""Graphwise KL loss (segment_reduce) on 8 trn2 NeuronCores.

Strategy (v2, bf16 + engine-balanced):
  Host:
    - Cast y_pred/y_true/weight to bf16 (tolerance 2e-2 >> bf16 error) and
      pre-transpose each [TILE_F, 128] chunk so that an SBUF column (one
      partition-dim stripe) holds 128 CONSECUTIVE elements of the original
      array.  This halves HBM traffic and lets the PE (tensor) engine do the
      block reductions across partitions.
  Device (per core, 4 double-buffered tiles of [128, 2048] bf16):
    sync  : HWDGE dma loads of yp/yt/w tiles + final store
    DVE   : pr = yt*w ; d = lp - lq ; e1 = pr*d        (bf16 2x mode)
    ACT   : lp = Ln(pr + 1e-37) ; lq = Ln(yp + 1e-8)   (+ warmup table load)
    PE    : per 128-column chunk: matmul(stationary=data chunk [128,128],
            moving=ones [128,1]) -> psum[:, col] = per-column sums
            = 128-element block sums for e1 and pr (psum [128, 128] f32)
    DVE   : single psum -> SBUF copy at the end; sync DMAs 64KB out.
  Host assembly (fp64): identical to v1 but with BLK=128; boundary partial
  sums recomputed from the bf16-cast inputs for consistency with the device.
    With S_g = max(B_g, EPS):  total = mean_g (A_g - B_g*ln(S_g)) / S_g.

  Raw Bass (no Tile): one inline sync wait max per instruction; standalone
  wait_ge everywhere; even/odd DMA semaphores per stream.
"""

import numpy as np
from ml_dtypes import bfloat16

N_TOTAL = 8388608
N_CORES = 8
N_LOCAL = N_TOTAL // N_CORES      # 1048576
P = 128
TILE_F = 2048                     # free dim of one macro tile
TILE_ELEMS = P * TILE_F           # 262144
N_TILES = N_LOCAL // TILE_ELEMS   # 4
BLK = 128                         # block = one transposed column
CHUNK = 128                       # stationary columns per matmul
CPT = TILE_F // CHUNK             # 16 chunks per tile
COLS = N_TILES * CPT              # 64 psum columns per array
N_BLOCKS_LOCAL = N_LOCAL // BLK   # 8192
EPS = 1e-8
TINY = 1e-37

_CACHE = {}


def _check_one_wait(nc):
    """Assert no non-EventSemaphore instruction carries more than one wait."""
    bad = []
    for f in nc.m.functions:
        for bb in f.blocks:
            for inst in bb.instructions:
                si = inst.sync_info
                if si and si.on_wait and len(si.on_wait) > 1:
                    if "EventSem" not in type(inst).__name__:
                        bad.append((type(inst).__name__, inst.name, len(si.on_wait)))
    assert not bad, f"multi-wait instructions remain: {bad}"


def _build_program():
    import concourse.bass as bass
    import concourse.mybir as mybir

    f32 = mybir.dt.float32
    bf16 = mybir.dt.bfloat16
    Ln = mybir.ActivationFunctionType.Ln

    nc = bass.Bass()

    # Const APs for the Ln biases (same mechanism Bass.__init__ uses for 0/1).
    for val in (TINY, EPS):
        ct = nc.alloc_sbuf_tensor(f"const-f32-{val}", [128, 1], f32)
        nc.gpsimd.memset(ct.ap(), val)
        nc.const_aps.aps[(f32, val)] = ct.ap()
    nc.all_engine_barrier()

    yp = nc.declare_dram_parameter("yp", [N_LOCAL], bf16, isOutput=False)
    yt = nc.declare_dram_parameter("yt", [N_LOCAL], bf16, isOutput=False)
    w = nc.declare_dram_parameter("w", [N_LOCAL], bf16, isOutput=False)
    o = nc.declare_dram_parameter("o", [P * 2 * COLS], f32, isOutput=True)

    yp3 = yp[:].rearrange("(t p f) -> t p f", p=P, f=TILE_F)
    yt3 = yt[:].rearrange("(t p f) -> t p f", p=P, f=TILE_F)
    w3 = w[:].rearrange("(t p f) -> t p f", p=P, f=TILE_F)
    o2 = o[:].rearrange("(p f) -> p f", p=P)

    def buf2(name, shape, dt):
        return [nc.alloc_sbuf_tensor(f"{name}{i}", shape, dt).ap() for i in range(2)]

    t_yp = buf2("t_yp", [P, TILE_F], bf16)
    t_yt = buf2("t_yt", [P, TILE_F], bf16)
    t_w = buf2("t_w", [P, TILE_F], bf16)
    t_pr = buf2("t_pr", [P, TILE_F], bf16)
    t_lp = buf2("t_lp", [P, TILE_F], bf16)
    t_lq = buf2("t_lq", [P, TILE_F], bf16)
    t_d = buf2("t_d", [P, TILE_F], bf16)
    t_e1 = buf2("t_e1", [P, TILE_F], bf16)
    out_sb = nc.alloc_sbuf_tensor("out_sb", [P, 2 * COLS], f32).ap()
    warm_sb = nc.alloc_sbuf_tensor("warm_sb", [P, 1], f32).ap()

    ps = nc.alloc_psum_tensor("ps", [P, 2 * COLS], f32).ap()

    ones_bf = nc.const_aps.aps[(bf16, 1.0)]   # [128, 1]
    zero_f32 = nc.const_aps.aps[(f32, 0.0)]   # [128, 1]

    # Even/odd semaphores per DMA stream: at most ONE DMA in flight per sem.
    s_yp = [nc.alloc_semaphore(f"s_yp{i}") for i in range(2)]
    s_yt = [nc.alloc_semaphore(f"s_yt{i}") for i in range(2)]
    s_w = [nc.alloc_semaphore(f"s_w{i}") for i in range(2)]
    s_act = nc.alloc_semaphore("s_act")   # +1 per ACT op (warm, lp, lq)
    s_dve = nc.alloc_semaphore("s_dve")   # +1 per DVE op
    s_pe = nc.alloc_semaphore("s_pe")     # +1 per finished tile of matmuls
    s_out = nc.alloc_semaphore("s_out")   # +16 for the output store

    # DVE op order (pr hoisted two tiles ahead for cross-engine overlap):
    #   pr0 pr1 [d,e1]0 pr2 [d,e1]1 pr3 [d,e1]2 [d,e1]3 copy
    dve_idx = {}
    n = 0
    order = [("pr", 0), ("pr", 1)]
    for t in range(N_TILES):
        order.append(("de", t))
        if t + 2 < N_TILES:
            order.append(("pr", t + 2))
    for kind, t in order:
        if kind == "pr":
            n += 1
            dve_idx[("pr", t)] = n
        else:
            n += 1
            dve_idx[("d", t)] = n
            n += 1
            dve_idx[("e1", t)] = n
    n_dve_total = n + 1  # + final psum copy
    # ACT indices: warm = 1, lp(t) = 2t+2, lq(t) = 2t+3

    with nc.Block() as block:

        @block.sync
        def _(s):
            for t in range(N_TILES):
                buf = t % 2
                if t >= 2:
                    # yp[buf] was read by lq(t-2); yt/w[buf] by pr(t-2)
                    s.wait_ge(s_act, 2 * (t - 2) + 3)
                    s.wait_ge(s_dve, dve_idx[("pr", t - 2)])
                s.dma_start(t_yp[buf], yp3[t, :, :]).then_inc(s_yp[buf], 16)
                s.dma_start(t_yt[buf], yt3[t, :, :]).then_inc(s_yt[buf], 16)
                s.dma_start(t_w[buf], w3[t, :, :]).then_inc(s_w[buf], 16)
            # final store after the psum copy landed in SBUF
            s.wait_ge(s_dve, n_dve_total)
            s.dma_start(o2, out_sb).then_inc(s_out, 16)
            s.wait_ge(s_out, 16)

        @block.scalar
        def _(a):
            # warmup: trigger the Ln table load during the first DMA fill
            a.activation(warm_sb, zero_f32, Ln, bias=EPS).then_inc(s_act, 1)
            for t in range(N_TILES):
                buf = t % 2
                # lp(t) needs DVE pr(t); also covers lp/lq[buf] slot reuse
                # (d(t-2) precedes pr(t) in DVE order)
                a.wait_ge(s_dve, dve_idx[("pr", t)])
                a.activation(t_lp[buf], t_pr[buf], Ln, bias=TINY).then_inc(s_act, 1)
                a.wait_ge(s_yp[buf], 16 * (t // 2 + 1))
                a.activation(t_lq[buf], t_yp[buf], Ln, bias=EPS).then_inc(s_act, 1)

        @block.vector
        def _(v):
            def emit_pr(t):
                buf = t % 2
                v.wait_ge(s_yt[buf], 16 * (t // 2 + 1))
                v.wait_ge(s_w[buf], 16 * (t // 2 + 1))
                if t >= 2:
                    # pr/e1[buf] were consumed by PE matmuls of tile t-2
                    v.wait_ge(s_pe, t - 1)
                v.tensor_mul(t_pr[buf], t_yt[buf], t_w[buf]).then_inc(s_dve, 1)

            def emit_de(t):
                buf = t % 2
                v.wait_ge(s_act, 2 * t + 3)  # lp(t), lq(t) done
                v.tensor_sub(t_d[buf], t_lp[buf], t_lq[buf]).then_inc(s_dve, 1)
                # same-engine RAW: DVE does not forward; explicit wait
                v.wait_ge(s_dve, dve_idx[("d", t)])
                v.tensor_mul(t_e1[buf], t_pr[buf], t_d[buf]).then_inc(s_dve, 1)

            for kind, t in order:
                if kind == "pr":
                    emit_pr(t)
                else:
                    emit_de(t)
            # exit psum once all matmuls are done
            v.wait_ge(s_pe, N_TILES)
            v.tensor_copy(out_sb, ps).then_inc(s_dve, 1)

        @block.tensor
        def _(te):
            for t in range(N_TILES):
                buf = t % 2
                te.wait_ge(s_dve, dve_idx[("e1", t)])
                for c in range(CPT):
                    col = t * CPT + c
                    sl = slice(c * CHUNK, (c + 1) * CHUNK)
                    te.matmul(ps[:, col:col + 1], t_e1[buf][:, sl], ones_bf,
                              start=True, stop=True)
                    mm = te.matmul(ps[:, COLS + col:COLS + col + 1],
                                   t_pr[buf][:, sl], ones_bf,
                                   start=True, stop=True)
                mm.then_inc(s_pe, 1)

    _check_one_wait(nc)
    return nc


def _get_program():
    if "nc" not in _CACHE:
        _CACHE["nc"] = _build_program()
    return _CACHE["nc"]


def _prep(x):
    """f32 [N_TOTAL] -> per-core bf16 arrays in the transposed tile layout:
    dram[t, p, f] = x[core_base + t*TILE_ELEMS + f*P + p]."""
    xb = np.ascontiguousarray(np.asarray(x), dtype=np.float32).astype(bfloat16)
    xt = xb.reshape(N_CORES, N_TILES, TILE_F, P).transpose(0, 1, 3, 2)
    return [np.ascontiguousarray(xt[k]).reshape(N_LOCAL) for k in range(N_CORES)], xb


def _run_device(yp_s, yt_s, w_s, trace=False):
    from concourse.bass_utils import run_bass_kernel_spmd

    nc = _get_program()
    in_maps = [
        {"yp": yp_s[k], "yt": yt_s[k], "w": w_s[k]} for k in range(N_CORES)
    ]
    res = run_bass_kernel_spmd(nc, in_maps, list(range(N_CORES)), trace=trace)
    bs1 = []
    bs2 = []
    for r in res.results:
        O = np.asarray(r["o"]).reshape(P, 2 * COLS)
        bs1.append(O[:, :COLS].T.ravel())   # block b = 128*col + m
        bs2.append(O[:, COLS:].T.ravel())
    return np.concatenate(bs1), np.concatenate(bs2), res


def kernel(y_pred, y_true, weight, segment_ptr, _trace=False):
    ptr = np.asarray(segment_ptr).astype(np.int64).reshape(-1)
    n = N_TOTAL
    G = ptr.shape[0] - 1

    yp_s, yp_b = _prep(y_pred)
    yt_s, yt_b = _prep(y_true)
    w_s, w_b = _prep(weight)

    bs1, bs2, res = _run_device(yp_s, yt_s, w_s, trace=_trace)
    _CACHE["last_res"] = res

    # ---- host assembly in fp64 ----
    pre1 = np.empty(bs1.shape[0] + 1)
    pre1[0] = 0.0
    np.cumsum(bs1, dtype=np.float64, out=pre1[1:])
    pre2 = np.empty(bs2.shape[0] + 1)
    pre2[0] = 0.0
    np.cumsum(bs2, dtype=np.float64, out=pre2[1:])

    ptrc = np.clip(ptr, 0, n)
    b_idx = ptrc // BLK
    r = ptrc - b_idx * BLK
    seg_off = np.concatenate([[0], np.cumsum(r)])
    tot = int(seg_off[-1])
    part1 = np.zeros(ptrc.shape[0])
    part2 = np.zeros(ptrc.shape[0])
    if tot > 0:
        idx = np.repeat(ptrc - r, r) + (np.arange(tot) - np.repeat(seg_off[:-1], r))
        # use the bf16-cast values for consistency with the device pass
        pr_h = (yt_b[idx].astype(np.float64) * w_b[idx].astype(np.float64))
        pr_h = pr_h.astype(bfloat16).astype(np.float64)
        e1_h = pr_h * (np.log(pr_h + TINY)
                       - np.log(yp_b[idx].astype(np.float64) + EPS))
        nz = r > 0
        red_idx = np.minimum(seg_off[:-1][nz], tot - 1).astype(np.int64)
        part1[nz] = np.add.reduceat(e1_h, red_idx)
        part2[nz] = np.add.reduceat(pr_h, red_idx)

    C1 = pre1[b_idx] + part1
    C2 = pre2[b_idx] + part2
    A = np.diff(C1)
    Bg = np.diff(C2)
    S = np.maximum(Bg, EPS)
    total = np.sum((A - Bg * np.log(S)) / S) / max(G, 1)
    return np.float32(total)
